# revision 12
# baseline (speedup 1.0000x reference)
"""DISCO S2 convolution (nn_DISCOBlock_57801669869705) on 8 Trainium2 NeuronCores.

out[b,o,to,q] = sum_{c,k} w[o,c,k] * sum_{w,p} psi[k,to,w,p] * x[b,c,ti[to,w],(p+q)%P]

Mapping (v2, M=128): two adjacent output latitude rows (r0, r1=r0+1) are fused
into one "unit" computed by a single PSUM accumulation chain. Their 10-row
input-latitude window union splits into 5 contraction pairs; for each active
longitude-shift tap (pair j, dp) one TensorE matmul accumulates
    acc[(r,o), (q,b)] += WPsi[(m,c), (r,o)].T @ xg[(m,c), (q+dp, b)]
with contraction over 128 partitions = (pair member m, in-channel c) and
M = 128 = (unit row r, out-channel o), N = (q,b) = 360, in bfloat16
(fp32 PSUM accumulate). WPsi = sum_k psi[k,to_r,w,dp] * weight[o,c,k] is a
host-side transform of the weight tensor; xg holds latitudinally gathered,
longitudinally haloed input rows.

Sharding: the 46 units (45 adjacent row pairs + row 90) are grouped into
6 slots of <=8 units; a slot's units run simultaneously on the 8 cores under
a shared per-slot tap template (union of the units' taps; absent taps get
zero coefficients). Basis values with negligible L2 mass (<=1e-5 of total)
are truncated, adding ~3e-3 relative error (well under the 2e-2 gate).
"""

import math
from functools import lru_cache

import numpy as np

B, C, O = 2, 64, 64
NLAT, P = 91, 180
NR, NPHI = 5, 6
K = (NR - 1) * NPHI + 1
HALF = 4
W = 2 * HALF + 1
NCORE = 8
NSLOT = 6
NJ = 5
TAIL = 1e-5     # truncated L2 mass fraction of psi
WP_CHUNK = 96   # steady-state taps per streamed weight-block DMA
WP_RAMP = (8, 16, 32, 64)  # graduated first chunks (fill the DMA pipe)


def _compute_psi():
    theta_cut = 4.0 * math.pi / (NLAT - 1)
    theta = np.pi * np.arange(NLAT) / (NLAT - 1)
    phi_in = 2.0 * np.pi * np.arange(P) / P
    offs = np.arange(-HALF, HALF + 1)
    ti_raw = np.arange(NLAT)[:, None] + offs[None, :]
    valid = (ti_raw >= 0) & (ti_raw < NLAT)
    ti_idx = np.clip(ti_raw, 0, NLAT - 1)
    to = theta[:, None, None]
    ti = theta[ti_idx][:, :, None]
    ph = phi_in[None, None, :]
    xx = np.cos(to) * np.sin(ti) * np.cos(ph) - np.sin(to) * np.cos(ti)
    yy = np.sin(ti) * np.sin(ph)
    zz = np.sin(to) * np.sin(ti) * np.cos(ph) + np.cos(to) * np.cos(ti)
    r = np.arccos(np.clip(zz, -1.0, 1.0))
    az = np.mod(np.arctan2(yy, xx), 2.0 * np.pi)
    dr = theta_cut / (NR - 1)
    dphi = 2.0 * np.pi / NPHI
    inside = (r <= theta_cut) & valid[:, :, None]
    psi = np.zeros((K,) + r.shape)
    psi[0] = np.where(inside, np.maximum(0.0, 1.0 - r / dr), 0.0)
    for ir in range(1, NR):
        rad = np.maximum(0.0, 1.0 - np.abs(r - ir * dr) / dr)
        for ip in range(NPHI):
            d = np.abs(np.mod(az - ip * dphi + np.pi, 2.0 * np.pi) - np.pi)
            ang = np.maximum(0.0, 1.0 - d / dphi)
            psi[1 + (ir - 1) * NPHI + ip] = np.where(inside, rad * ang, 0.0)
    quad = np.sin(theta) * (np.pi / (NLAT - 1)) * (2.0 * np.pi / P)
    psi = psi * quad[ti_idx][None, :, :, None]
    # truncate triples carrying <= TAIL of the total L2 mass
    G = (psi ** 2).sum(axis=0)
    act = G > 0
    vals = np.sort(G[act])
    cum = np.cumsum(vals)
    idx = np.searchsorted(cum, TAIL * G.sum())
    if idx > 0:
        psi[:, (G <= vals[idx - 1]) & act] = 0.0
    return psi.astype(np.float32), ti_idx.astype(np.int32)


def _unit_groups(active, unit):
    """Optimal grouping of a unit's active input rows into <=NJ pairs/singles
    minimizing total |dp-activity union|. Returns 5 (tis_tuple, dpset)."""
    dsets = {}
    for ti in range(max(0, unit[0] - HALF), min(NLAT - 1, unit[-1] + HALF) + 1):
        s = set()
        for r in unit:
            w = ti - r + HALF
            if 0 <= w < W:
                s |= set(np.nonzero(active[r, w])[0].tolist())
        if s:
            dsets[ti] = frozenset(s)
    tis = sorted(dsets)
    n = len(tis)

    @lru_cache(maxsize=None)
    def f(mask):
        if mask == 0:
            return 0, ()
        a = (mask & -mask).bit_length() - 1
        rest = mask & ~(1 << a)
        c, pl = f(rest)
        best = (len(dsets[tis[a]]) + c, pl + ((tis[a],),))
        for b in range(a + 1, n):
            if rest >> b & 1:
                c, pl = f(rest & ~(1 << b))
                cc = len(dsets[tis[a]] | dsets[tis[b]]) + c
                if cc < best[0]:
                    best = (cc, pl + ((tis[a], tis[b]),))
        return best

    cost, pl = f((1 << n) - 1)
    f.cache_clear()
    assert len(pl) <= NJ, (unit, pl)
    groups = [(g, frozenset().union(*[dsets[t] for t in g])) for g in pl]
    groups.sort(key=lambda x: -len(x[1]))
    while len(groups) < NJ:
        groups.append(((), frozenset()))
    return groups, cost


def _build_plan():
    psi, ti_idx = _compute_psi()
    active = (psi != 0).any(axis=0)  # [To, W, P]
    dpval = np.where(np.arange(P) < P // 2, np.arange(P), np.arange(P) - P)

    units = [(i, i + 1) for i in range(0, NLAT - 1, 2)] + [(NLAT - 1,)]
    infos = []
    for u in units:
        groups, cost = _unit_groups(active, u)
        infos.append((u, groups, cost))
    infos.sort(key=lambda x: -x[2])

    def slot_cost(sub):
        tot = 0
        for j in range(NJ):
            s = set()
            for _, groups, _ in sub:
                s |= groups[j][1]
            tot += len(s)
        return tot

    n = len(infos)
    INF = 10 ** 12
    cost = {}
    for i0 in range(n):
        for i1 in range(i0 + 1, min(i0 + NCORE, n) + 1):
            cost[(i0, i1)] = slot_cost(infos[i0:i1])
    dp = [[INF] * (NSLOT + 1) for _ in range(n + 1)]
    par = [[0] * (NSLOT + 1) for _ in range(n + 1)]
    dp[0][0] = 0
    for j in range(1, NSLOT + 1):
        for i in range(1, n + 1):
            for i0 in range(max(0, i - NCORE), i):
                v = dp[i0][j - 1] + cost[(i0, i)]
                if v < dp[i][j]:
                    dp[i][j] = v
                    par[i][j] = i0
    bounds = []
    i = n
    for j in range(NSLOT, 0, -1):
        i0 = par[i][j]
        bounds.append((i0, i))
        i = i0
    bounds = bounds[::-1]

    slot_units = []      # [s][core] -> (unit rows, groups)
    templates = []       # [s] -> list of (j, dp)
    jspans = []          # [s][j] -> (h, qpad)
    for (i0, i1) in bounds:
        sub = infos[i0:i1]
        slot_units.append([(u, groups) for (u, groups, _) in sub])
        taps = []
        spans = []
        for j in range(NJ):
            dps = set()
            for _, groups, _ in sub:
                dps |= groups[j][1]
            dps = sorted(dpval[p] for p in dps)
            taps += [(j, d) for d in dps]
            if dps:
                h = max(-dps[0], 0)
                qpad = P + h + max(dps[-1], 0)
            else:
                h, qpad = 0, 0
            spans.append((h, qpad))
        templates.append(taps)
        jspans.append(spans)

    # xg column offsets per (s, j)
    xg_off = []
    col = 0
    for s in range(NSLOT):
        row = []
        for j in range(NJ):
            row.append(col)
            col += B * jspans[s][j][1]
        xg_off.append(row)
    return dict(psi=psi, ti_idx=ti_idx, slot_units=slot_units,
                templates=templates, jspans=jspans, xg_off=xg_off,
                xg_cols=col, t_total=sum(len(t) for t in templates))


_PLAN = None
_NC = None


def _get_plan():
    global _PLAN
    if _PLAN is None:
        _PLAN = _build_plan()
    return _PLAN


def _build_nc(plan):
    import concourse.bacc as bacc
    import concourse.mybir as mybir
    import concourse.tile as tile

    f32 = mybir.dt.float32
    bf16 = mybir.dt.bfloat16

    templates = plan["templates"]
    jspans = plan["jspans"]
    xg_off = plan["xg_off"]
    XG_COLS = plan["xg_cols"]
    T = plan["t_total"]

    nc = bacc.Bacc("TRN2", target_bir_lowering=False, debug=False,
                   num_devices=NCORE)
    xg_d = nc.declare_dram_parameter("xg", [128, XG_COLS], bf16, isOutput=False)
    wp_d = nc.declare_dram_parameter("wp", [128, T * 128], bf16, isOutput=False)
    out_d = nc.declare_dram_parameter("out", [128, NSLOT * B * P], f32,
                                      isOutput=True)

    with tile.TileContext(nc) as tc:
        with (
            tc.tile_pool(name="xg", bufs=1) as xgp,
            tc.tile_pool(name="wp", bufs=3) as wpp,
            tc.tile_pool(name="ps", bufs=2, space="PSUM") as psp,
            tc.tile_pool(name="outp", bufs=1) as outp,
            tc.tile_pool(name="warmp", bufs=1, space="PSUM") as warmp,
        ):
            # PE warm-up: dummy matmuls on a zeroed scratch tile while the
            # first weight chunk is still in flight. Keeps the HAM clock
            # gate at full rate so the first real matmuls aren't throttled.
            warm = xgp.tile([128, B * P], bf16, tag="warm")
            nc.vector.memzero(warm[:])
            wacc = warmp.tile([128, B * P], f32)
            NWARM = 36
            for i in range(NWARM):
                nc.tensor.matmul(wacc[:], warm[:, :128], warm[:],
                                 start=(i == 0), stop=(i == NWARM - 1))
            # xg tiles on the scalar HWDGE queue so inputs load in
            # parallel with the weight chunks (sync/HWDGE). Only slot 0's
            # xg is fetched upfront; slot s+1's fetch is issued at the
            # start of slot s so the early HBM bandwidth goes to the
            # weight-chunk ramp.
            xg_ts = []
            xg_cols_s = []
            for s in range(NSLOT):
                cols = sum(B * jspans[s][j][1] for j in range(NJ))
                seg = xgp.tile([128, cols], bf16, tag=f"xg{s}")
                xg_ts.append(seg)
                xg_cols_s.append(cols)
            nc.scalar.dma_start(
                xg_ts[0][:], xg_d[:, xg_off[0][0]: xg_off[0][0] + xg_cols_s[0]])
            out_t = outp.tile([128, NSLOT * B * P], f32)

            # weight chunk boundaries: graduated first chunks to fill the
            # DMA pipeline, then steady WP_CHUNK-tap chunks
            bounds = [0]
            for r in WP_RAMP:
                if bounds[-1] + r < T:
                    bounds.append(bounds[-1] + r)
            while bounds[-1] < T:
                bounds.append(min(T, bounds[-1] + WP_CHUNK))
            chunk_of = []
            for ci_ in range(len(bounds) - 1):
                chunk_of += [(ci_, bounds[ci_])] * (bounds[ci_ + 1] - bounds[ci_])

            tg = 0
            wp_t = None
            for s in range(NSLOT):
                taps = templates[s]
                acc = psp.tile([128, B * P], f32)
                if s + 1 < NSLOT:
                    nc.scalar.dma_start(
                        xg_ts[s + 1][:],
                        xg_d[:, xg_off[s + 1][0]:
                             xg_off[s + 1][0] + xg_cols_s[s + 1]])
                for i, (j, dp) in enumerate(taps):
                    cidx, cbase = chunk_of[tg]
                    if tg == cbase:
                        cols = (bounds[cidx + 1] - cbase) * 128
                        wp_t = wpp.tile([128, WP_CHUNK * 128], bf16, tag="wp")
                        nc.sync.dma_start(
                            wp_t[:, :cols], wp_d[:, cbase * 128: cbase * 128 + cols])
                    lhsT = wp_t[:, (tg - cbase) * 128:(tg - cbase + 1) * 128]
                    h = jspans[s][j][0]
                    base = xg_off[s][j] - xg_off[s][0]
                    xv = xg_ts[s]
                    rhs = xv[:, base + B * (h + dp): base + B * (h + dp + P)]
                    nc.tensor.matmul(acc[:], lhsT, rhs,
                                     start=(i == 0), stop=(i == len(taps) - 1))
                    tg += 1
                nc.vector.tensor_copy(
                    out_t[:, s * B * P:(s + 1) * B * P], acc[:])
                nc.scalar.dma_start(
                    out_d[:, s * B * P:(s + 1) * B * P],
                    out_t[:, s * B * P:(s + 1) * B * P])

    nc.move_matmul_waits_to_ldweights()
    nc.compile()
    return nc


def _get_nc():
    global _NC
    if _NC is None:
        _NC = _build_nc(_get_plan())
    return _NC


def _build_core_inputs(plan, x, weight):
    import ml_dtypes

    psi = plan["psi"]
    slot_units = plan["slot_units"]
    templates = plan["templates"]
    jspans = plan["jspans"]
    xg_off = plan["xg_off"]
    XG_COLS = plan["xg_cols"]
    T = plan["t_total"]

    # per-tap basis coefficients: coef[core, tap, m, r, k]
    coef = np.zeros((NCORE, T, 2, 2, K), dtype=np.float32)
    tg = 0
    for s in range(NSLOT):
        units = slot_units[s]
        for (j, dp) in templates[s]:
            p = dp % P
            for core in range(min(NCORE, len(units))):
                u, groups = units[core]
                tis = groups[j][0]
                for m, ti in enumerate(tis):
                    for r, row in enumerate(u):
                        w = ti - row + HALF
                        if 0 <= w < W:
                            coef[core, tg, m, r] = psi[:, row, w, p]
            tg += 1
    wk = np.ascontiguousarray(weight.transpose(2, 1, 0)).reshape(K, C, O)
    wps = []
    for n in range(NCORE):
        # [m, c, t, r, o] -> [128, T*128]
        wp = np.einsum("tmrk,kco->mctro", coef[n], wk, optimize=True)
        wps.append(np.ascontiguousarray(
            wp.reshape(128, T * 128)).astype(ml_dtypes.bfloat16))

    xgs = []
    for core in range(NCORE):
        xg = np.zeros((128, XG_COLS), dtype=np.float32)
        for s in range(NSLOT):
            units = slot_units[s]
            if core >= len(units):
                continue
            u, groups = units[core]
            for j in range(NJ):
                tis = groups[j][0]
                h, qp = jspans[s][j]
                if not qp:
                    continue
                qq = (np.arange(qp) - h) % P
                for m, ti in enumerate(tis):
                    blk = x[:, :, ti, :][:, :, qq]  # [b, c, qp]
                    xg[m * 64:(m + 1) * 64,
                       xg_off[s][j]: xg_off[s][j] + B * qp] = (
                        blk.transpose(1, 2, 0).reshape(C, qp * B))
        xgs.append(xg.astype(ml_dtypes.bfloat16))
    return xgs, wps


def kernel(x, weight):
    from concourse.bass_utils import run_bass_kernel_spmd

    x = np.ascontiguousarray(np.asarray(x, dtype=np.float32))
    weight = np.ascontiguousarray(np.asarray(weight, dtype=np.float32))
    plan = _get_plan()
    nc = _get_nc()
    xgs, wps = _build_core_inputs(plan, x, weight)
    in_maps = [{"xg": xgs[i], "wp": wps[i]} for i in range(NCORE)]
    res = run_bass_kernel_spmd(nc, in_maps, list(range(NCORE)))

    out = np.zeros((B, O, NLAT, P), dtype=np.float32)
    slot_units = plan["slot_units"]
    for core in range(NCORE):
        oc = np.asarray(res.results[core]["out"]).reshape(128, NSLOT, P, B)
        for s in range(NSLOT):
            units = slot_units[s]
            if core >= len(units):
                continue
            u, _ = units[core]
            for r, row in enumerate(u):
                out[:, :, row, :] = oc[r * 64:(r + 1) * 64, s].transpose(2, 0, 1)
    return out


def _numpy_sim(x, weight):
    """Host replica of the device program (for validation)."""
    plan = _get_plan()
    xgs, wps = _build_core_inputs(plan, x, weight)
    templates = plan["templates"]
    jspans = plan["jspans"]
    xg_off = plan["xg_off"]
    slot_units = plan["slot_units"]
    out = np.zeros((B, O, NLAT, P), dtype=np.float32)
    for core in range(NCORE):
        xg = xgs[core].astype(np.float32)
        wp = wps[core].astype(np.float32)
        tg = 0
        oc = np.zeros((128, NSLOT, P, B), dtype=np.float32)
        for s in range(NSLOT):
            acc = np.zeros((128, P * B), dtype=np.float32)
            for (j, dp) in templates[s]:
                lhsT = wp[:, tg * 128:(tg + 1) * 128]
                h = jspans[s][j][0]
                base = xg_off[s][j]
                rhs = xg[:, base + B * (h + dp): base + B * (h + dp + P)]
                acc += lhsT.T @ rhs
                tg += 1
            oc[:, s] = acc.reshape(128, P, B)
        units_all = slot_units
        for s in range(NSLOT):
            units = units_all[s]
            if core >= len(units):
                continue
            u, _ = units[core]
            for r, row in enumerate(u):
                out[:, :, row, :] = oc[r * 64:(r + 1) * 64, s].transpose(2, 0, 1)
    return out


if __name__ == "__main__":
    plan = _get_plan()
    print("t_total:", plan["t_total"], "xg_cols:", plan["xg_cols"],
          "xg MB (bf16):", plan["xg_cols"] * 128 * 2 / 1e6,
          "wp MB (bf16):", plan["t_total"] * 128 * 128 * 2 / 1e6)
    d = np.load("/tmp/ref_io.npz")
    got = _numpy_sim(d["x"], d["weight"])
    exp = d["expected"]
    rel = np.linalg.norm((got - exp).ravel()) / np.linalg.norm(exp.ravel())
    print("numpy-sim rel err:", rel)


# revision 14
# speedup vs baseline: 1.0463x; 1.0463x over previous
"""DISCO S2 convolution (nn_DISCOBlock_57801669869705) on 8 Trainium2 NeuronCores.

out[b,o,to,q] = sum_{c,k} w[o,c,k] * sum_{w,p} psi[k,to,w,p] * x[b,c,ti[to,w],(p+q)%P]

Mapping (v2, M=128): two adjacent output latitude rows (r0, r1=r0+1) are fused
into one "unit" computed by a single PSUM accumulation chain. Their 10-row
input-latitude window union splits into 5 contraction pairs; for each active
longitude-shift tap (pair j, dp) one TensorE matmul accumulates
    acc[(r,o), (q,b)] += WPsi[(m,c), (r,o)].T @ xg[(m,c), (q+dp, b)]
with contraction over 128 partitions = (pair member m, in-channel c) and
M = 128 = (unit row r, out-channel o), N = (q,b) = 360, in bfloat16
(fp32 PSUM accumulate). WPsi = sum_k psi[k,to_r,w,dp] * weight[o,c,k] is a
host-side transform of the weight tensor; xg holds latitudinally gathered,
longitudinally haloed input rows.

Sharding: the 46 units (45 adjacent row pairs + row 90) are grouped into
6 slots of <=8 units; a slot's units run simultaneously on the 8 cores under
a shared per-slot tap template (union of the units' taps; absent taps get
zero coefficients). Basis values with negligible L2 mass (<=1e-5 of total)
are truncated, adding ~3e-3 relative error (well under the 2e-2 gate).
"""

import math
from functools import lru_cache

import numpy as np

B, C, O = 2, 64, 64
NLAT, P = 91, 180
NR, NPHI = 5, 6
K = (NR - 1) * NPHI + 1
HALF = 4
W = 2 * HALF + 1
NCORE = 8
NSLOT = 6
NJ = 5
TAIL = 1e-5     # truncated L2 mass fraction of psi
WP_CHUNK = 96   # steady-state taps per streamed weight-block DMA
WP_RAMP = (8, 16, 32, 64)  # graduated first chunks (fill the DMA pipe)


def _compute_psi():
    theta_cut = 4.0 * math.pi / (NLAT - 1)
    theta = np.pi * np.arange(NLAT) / (NLAT - 1)
    phi_in = 2.0 * np.pi * np.arange(P) / P
    offs = np.arange(-HALF, HALF + 1)
    ti_raw = np.arange(NLAT)[:, None] + offs[None, :]
    valid = (ti_raw >= 0) & (ti_raw < NLAT)
    ti_idx = np.clip(ti_raw, 0, NLAT - 1)
    to = theta[:, None, None]
    ti = theta[ti_idx][:, :, None]
    ph = phi_in[None, None, :]
    xx = np.cos(to) * np.sin(ti) * np.cos(ph) - np.sin(to) * np.cos(ti)
    yy = np.sin(ti) * np.sin(ph)
    zz = np.sin(to) * np.sin(ti) * np.cos(ph) + np.cos(to) * np.cos(ti)
    r = np.arccos(np.clip(zz, -1.0, 1.0))
    az = np.mod(np.arctan2(yy, xx), 2.0 * np.pi)
    dr = theta_cut / (NR - 1)
    dphi = 2.0 * np.pi / NPHI
    inside = (r <= theta_cut) & valid[:, :, None]
    psi = np.zeros((K,) + r.shape)
    psi[0] = np.where(inside, np.maximum(0.0, 1.0 - r / dr), 0.0)
    for ir in range(1, NR):
        rad = np.maximum(0.0, 1.0 - np.abs(r - ir * dr) / dr)
        for ip in range(NPHI):
            d = np.abs(np.mod(az - ip * dphi + np.pi, 2.0 * np.pi) - np.pi)
            ang = np.maximum(0.0, 1.0 - d / dphi)
            psi[1 + (ir - 1) * NPHI + ip] = np.where(inside, rad * ang, 0.0)
    quad = np.sin(theta) * (np.pi / (NLAT - 1)) * (2.0 * np.pi / P)
    psi = psi * quad[ti_idx][None, :, :, None]
    # truncate triples carrying <= TAIL of the total L2 mass
    G = (psi ** 2).sum(axis=0)
    act = G > 0
    vals = np.sort(G[act])
    cum = np.cumsum(vals)
    idx = np.searchsorted(cum, TAIL * G.sum())
    if idx > 0:
        psi[:, (G <= vals[idx - 1]) & act] = 0.0
    return psi.astype(np.float32), ti_idx.astype(np.int32)


def _unit_groups(active, unit):
    """Optimal grouping of a unit's active input rows into <=NJ pairs/singles
    minimizing total |dp-activity union|. Returns 5 (tis_tuple, dpset)."""
    dsets = {}
    for ti in range(max(0, unit[0] - HALF), min(NLAT - 1, unit[-1] + HALF) + 1):
        s = set()
        for r in unit:
            w = ti - r + HALF
            if 0 <= w < W:
                s |= set(np.nonzero(active[r, w])[0].tolist())
        if s:
            dsets[ti] = frozenset(s)
    tis = sorted(dsets)
    n = len(tis)

    @lru_cache(maxsize=None)
    def f(mask):
        if mask == 0:
            return 0, ()
        a = (mask & -mask).bit_length() - 1
        rest = mask & ~(1 << a)
        c, pl = f(rest)
        best = (len(dsets[tis[a]]) + c, pl + ((tis[a],),))
        for b in range(a + 1, n):
            if rest >> b & 1:
                c, pl = f(rest & ~(1 << b))
                cc = len(dsets[tis[a]] | dsets[tis[b]]) + c
                if cc < best[0]:
                    best = (cc, pl + ((tis[a], tis[b]),))
        return best

    cost, pl = f((1 << n) - 1)
    f.cache_clear()
    assert len(pl) <= NJ, (unit, pl)
    groups = [(g, frozenset().union(*[dsets[t] for t in g])) for g in pl]
    groups.sort(key=lambda x: -len(x[1]))
    while len(groups) < NJ:
        groups.append(((), frozenset()))
    return groups, cost


def _build_plan():
    psi, ti_idx = _compute_psi()
    active = (psi != 0).any(axis=0)  # [To, W, P]
    dpval = np.where(np.arange(P) < P // 2, np.arange(P), np.arange(P) - P)

    units = [(i, i + 1) for i in range(0, NLAT - 1, 2)] + [(NLAT - 1,)]
    infos = []
    for u in units:
        groups, cost = _unit_groups(active, u)
        infos.append((u, groups, cost))
    infos.sort(key=lambda x: -x[2])

    def slot_cost(sub):
        tot = 0
        for j in range(NJ):
            s = set()
            for _, groups, _ in sub:
                s |= groups[j][1]
            tot += len(s)
        return tot

    n = len(infos)
    INF = 10 ** 12
    cost = {}
    for i0 in range(n):
        for i1 in range(i0 + 1, min(i0 + NCORE, n) + 1):
            cost[(i0, i1)] = slot_cost(infos[i0:i1])
    dp = [[INF] * (NSLOT + 1) for _ in range(n + 1)]
    par = [[0] * (NSLOT + 1) for _ in range(n + 1)]
    dp[0][0] = 0
    for j in range(1, NSLOT + 1):
        for i in range(1, n + 1):
            for i0 in range(max(0, i - NCORE), i):
                v = dp[i0][j - 1] + cost[(i0, i)]
                if v < dp[i][j]:
                    dp[i][j] = v
                    par[i][j] = i0
    bounds = []
    i = n
    for j in range(NSLOT, 0, -1):
        i0 = par[i][j]
        bounds.append((i0, i))
        i = i0
    # ascending tap count: small slots first so the big polar slot's xg
    # loads late, leaving the early HBM bandwidth to the wp-chunk ramp
    # (infos is sorted by descending unit cost, so reversed bounds order
    # is ascending slot cost)

    slot_units = []      # [s][core] -> (unit rows, groups)
    templates = []       # [s] -> list of (j, dp)
    jspans = []          # [s][j] -> (h, qpad)
    for (i0, i1) in bounds:
        sub = infos[i0:i1]
        slot_units.append([(u, groups) for (u, groups, _) in sub])
        taps = []
        spans = []
        for j in range(NJ):
            dps = set()
            for _, groups, _ in sub:
                dps |= groups[j][1]
            dps = sorted(dpval[p] for p in dps)
            taps += [(j, d) for d in dps]
            if dps:
                h = max(-dps[0], 0)
                qpad = P + h + max(dps[-1], 0)
            else:
                h, qpad = 0, 0
            spans.append((h, qpad))
        templates.append(taps)
        jspans.append(spans)

    # xg column offsets per (s, j)
    xg_off = []
    col = 0
    for s in range(NSLOT):
        row = []
        for j in range(NJ):
            row.append(col)
            col += B * jspans[s][j][1]
        xg_off.append(row)
    return dict(psi=psi, ti_idx=ti_idx, slot_units=slot_units,
                templates=templates, jspans=jspans, xg_off=xg_off,
                xg_cols=col, t_total=sum(len(t) for t in templates))


_PLAN = None
_NC = None


def _get_plan():
    global _PLAN
    if _PLAN is None:
        _PLAN = _build_plan()
    return _PLAN


def _build_nc(plan):
    import concourse.bacc as bacc
    import concourse.mybir as mybir
    import concourse.tile as tile

    f32 = mybir.dt.float32
    bf16 = mybir.dt.bfloat16

    templates = plan["templates"]
    jspans = plan["jspans"]
    xg_off = plan["xg_off"]
    XG_COLS = plan["xg_cols"]
    T = plan["t_total"]

    nc = bacc.Bacc("TRN2", target_bir_lowering=False, debug=False,
                   num_devices=NCORE)
    xg_d = nc.declare_dram_parameter("xg", [128, XG_COLS], bf16, isOutput=False)
    wp_d = nc.declare_dram_parameter("wp", [128, T * 128], bf16, isOutput=False)
    out_d = nc.declare_dram_parameter("out", [128, NSLOT * B * P], f32,
                                      isOutput=True)

    with tile.TileContext(nc) as tc:
        with (
            tc.tile_pool(name="xg", bufs=1) as xgp,
            tc.tile_pool(name="wp", bufs=3) as wpp,
            tc.tile_pool(name="ps", bufs=2, space="PSUM") as psp,
            tc.tile_pool(name="outp", bufs=1) as outp,
        ):
            # xg tiles on the scalar HWDGE queue so inputs load in
            # parallel with the weight chunks (sync/HWDGE). Only slot 0's
            # xg is fetched upfront; slot s+1's fetch is issued at the
            # start of slot s so the early HBM bandwidth goes to the
            # weight-chunk ramp.
            xg_ts = []
            xg_cols_s = []
            for s in range(NSLOT):
                cols = sum(B * jspans[s][j][1] for j in range(NJ))
                seg = xgp.tile([128, cols], bf16, tag=f"xg{s}")
                xg_ts.append(seg)
                xg_cols_s.append(cols)
            nc.scalar.dma_start(
                xg_ts[0][:], xg_d[:, xg_off[0][0]: xg_off[0][0] + xg_cols_s[0]])
            out_t = outp.tile([128, NSLOT * B * P], f32)

            # weight chunk boundaries: graduated first chunks to fill the
            # DMA pipeline, then steady WP_CHUNK-tap chunks
            bounds = [0]
            for r in WP_RAMP:
                if bounds[-1] + r < T:
                    bounds.append(bounds[-1] + r)
            while bounds[-1] < T:
                bounds.append(min(T, bounds[-1] + WP_CHUNK))
            chunk_of = []
            for ci_ in range(len(bounds) - 1):
                chunk_of += [(ci_, bounds[ci_])] * (bounds[ci_ + 1] - bounds[ci_])

            tg = 0
            wp_t = None
            for s in range(NSLOT):
                taps = templates[s]
                acc = psp.tile([128, B * P], f32)
                if s + 1 < NSLOT:
                    nc.scalar.dma_start(
                        xg_ts[s + 1][:],
                        xg_d[:, xg_off[s + 1][0]:
                             xg_off[s + 1][0] + xg_cols_s[s + 1]])
                for i, (j, dp) in enumerate(taps):
                    cidx, cbase = chunk_of[tg]
                    if tg == cbase:
                        cols = (bounds[cidx + 1] - cbase) * 128
                        wp_t = wpp.tile([128, WP_CHUNK * 128], bf16, tag="wp")
                        nc.sync.dma_start(
                            wp_t[:, :cols], wp_d[:, cbase * 128: cbase * 128 + cols])
                    lhsT = wp_t[:, (tg - cbase) * 128:(tg - cbase + 1) * 128]
                    h = jspans[s][j][0]
                    base = xg_off[s][j] - xg_off[s][0]
                    xv = xg_ts[s]
                    rhs = xv[:, base + B * (h + dp): base + B * (h + dp + P)]
                    nc.tensor.matmul(acc[:], lhsT, rhs,
                                     start=(i == 0), stop=(i == len(taps) - 1))
                    tg += 1
                nc.vector.tensor_copy(
                    out_t[:, s * B * P:(s + 1) * B * P], acc[:])
                nc.scalar.dma_start(
                    out_d[:, s * B * P:(s + 1) * B * P],
                    out_t[:, s * B * P:(s + 1) * B * P])

    nc.move_matmul_waits_to_ldweights()
    nc.compile()
    return nc


def _get_nc():
    global _NC
    if _NC is None:
        _NC = _build_nc(_get_plan())
    return _NC


def _build_core_inputs(plan, x, weight):
    import ml_dtypes

    psi = plan["psi"]
    slot_units = plan["slot_units"]
    templates = plan["templates"]
    jspans = plan["jspans"]
    xg_off = plan["xg_off"]
    XG_COLS = plan["xg_cols"]
    T = plan["t_total"]

    # per-tap basis coefficients: coef[core, tap, m, r, k]
    coef = np.zeros((NCORE, T, 2, 2, K), dtype=np.float32)
    tg = 0
    for s in range(NSLOT):
        units = slot_units[s]
        for (j, dp) in templates[s]:
            p = dp % P
            for core in range(min(NCORE, len(units))):
                u, groups = units[core]
                tis = groups[j][0]
                for m, ti in enumerate(tis):
                    for r, row in enumerate(u):
                        w = ti - row + HALF
                        if 0 <= w < W:
                            coef[core, tg, m, r] = psi[:, row, w, p]
            tg += 1
    wk = np.ascontiguousarray(weight.transpose(2, 1, 0)).reshape(K, C, O)
    wps = []
    for n in range(NCORE):
        # [m, c, t, r, o] -> [128, T*128]
        wp = np.einsum("tmrk,kco->mctro", coef[n], wk, optimize=True)
        wps.append(np.ascontiguousarray(
            wp.reshape(128, T * 128)).astype(ml_dtypes.bfloat16))

    xgs = []
    for core in range(NCORE):
        xg = np.zeros((128, XG_COLS), dtype=np.float32)
        for s in range(NSLOT):
            units = slot_units[s]
            if core >= len(units):
                continue
            u, groups = units[core]
            for j in range(NJ):
                tis = groups[j][0]
                h, qp = jspans[s][j]
                if not qp:
                    continue
                qq = (np.arange(qp) - h) % P
                for m, ti in enumerate(tis):
                    blk = x[:, :, ti, :][:, :, qq]  # [b, c, qp]
                    xg[m * 64:(m + 1) * 64,
                       xg_off[s][j]: xg_off[s][j] + B * qp] = (
                        blk.transpose(1, 2, 0).reshape(C, qp * B))
        xgs.append(xg.astype(ml_dtypes.bfloat16))
    return xgs, wps


def kernel(x, weight):
    from concourse.bass_utils import run_bass_kernel_spmd

    x = np.ascontiguousarray(np.asarray(x, dtype=np.float32))
    weight = np.ascontiguousarray(np.asarray(weight, dtype=np.float32))
    plan = _get_plan()
    nc = _get_nc()
    xgs, wps = _build_core_inputs(plan, x, weight)
    in_maps = [{"xg": xgs[i], "wp": wps[i]} for i in range(NCORE)]
    res = run_bass_kernel_spmd(nc, in_maps, list(range(NCORE)))

    out = np.zeros((B, O, NLAT, P), dtype=np.float32)
    slot_units = plan["slot_units"]
    for core in range(NCORE):
        oc = np.asarray(res.results[core]["out"]).reshape(128, NSLOT, P, B)
        for s in range(NSLOT):
            units = slot_units[s]
            if core >= len(units):
                continue
            u, _ = units[core]
            for r, row in enumerate(u):
                out[:, :, row, :] = oc[r * 64:(r + 1) * 64, s].transpose(2, 0, 1)
    return out


def _numpy_sim(x, weight):
    """Host replica of the device program (for validation)."""
    plan = _get_plan()
    xgs, wps = _build_core_inputs(plan, x, weight)
    templates = plan["templates"]
    jspans = plan["jspans"]
    xg_off = plan["xg_off"]
    slot_units = plan["slot_units"]
    out = np.zeros((B, O, NLAT, P), dtype=np.float32)
    for core in range(NCORE):
        xg = xgs[core].astype(np.float32)
        wp = wps[core].astype(np.float32)
        tg = 0
        oc = np.zeros((128, NSLOT, P, B), dtype=np.float32)
        for s in range(NSLOT):
            acc = np.zeros((128, P * B), dtype=np.float32)
            for (j, dp) in templates[s]:
                lhsT = wp[:, tg * 128:(tg + 1) * 128]
                h = jspans[s][j][0]
                base = xg_off[s][j]
                rhs = xg[:, base + B * (h + dp): base + B * (h + dp + P)]
                acc += lhsT.T @ rhs
                tg += 1
            oc[:, s] = acc.reshape(128, P, B)
        units_all = slot_units
        for s in range(NSLOT):
            units = units_all[s]
            if core >= len(units):
                continue
            u, _ = units[core]
            for r, row in enumerate(u):
                out[:, :, row, :] = oc[r * 64:(r + 1) * 64, s].transpose(2, 0, 1)
    return out


if __name__ == "__main__":
    plan = _get_plan()
    print("t_total:", plan["t_total"], "xg_cols:", plan["xg_cols"],
          "xg MB (bf16):", plan["xg_cols"] * 128 * 2 / 1e6,
          "wp MB (bf16):", plan["t_total"] * 128 * 128 * 2 / 1e6)
    d = np.load("/tmp/ref_io.npz")
    got = _numpy_sim(d["x"], d["weight"])
    exp = d["expected"]
    rel = np.linalg.norm((got - exp).ravel()) / np.linalg.norm(exp.ravel())
    print("numpy-sim rel err:", rel)
